# revision 2
# baseline (speedup 1.0000x reference)
"""Causal depthwise Conv1d (K=4) for Trainium2, 8 NeuronCores — xbar design.

Problem: x (B=8, L=4096, D=1024) f32, w (D, 1, 4), b (D,)
  y[n, l, d] = sum_k w[d, 0, k] * x[n, l - 3 + k, d] + b[d]   (zero pad l<0)

Sharding: data-parallel over batch — core i computes batch item i.

Per-core design (bf16 I/O):
  1. In: per d-group g (8 groups of 128 channels), XBAR DMA transposes
     (16x128 tiles, no PE involvement) land x directly as transposed
     whole-L SBUF tiles xt_g[128_d, 3+4096_l] (3 zero cols = causal halo).
     Chunked (512/1024-l pieces) so compute starts ~3.6us in.
  2. MAC for groups 0..6: natural-layout bf16 diag matmuls (4 taps per
     [128_l, 128_d] block) accumulating in PSUM; bias folded into the
     DVE tensor_tensor evacuation via a partition-broadcast bias band.
  3. Group 7 computes elementwise in transposed space (DVE tensor_scalar
     at 4x bf16 + DVE/Pool adds), PE transposes back, ACT evacuates.
  4. Per-superblock out DMAs (2KB descriptors); per-t on the last one.

Cost model: DMA is the pacer (~52us: 28.7 xbar-in + 23.3 out); PE ~50us,
DVE ~45us, ACT/Pool light. Numerics: bf16 in/compute/out.
"""

import sys
import types

import numpy as np

try:  # the NTFF profile hook module is absent in some containers
    import antenv.axon_hooks  # noqa: F401
except Exception:
    _stub = types.ModuleType("antenv.axon_hooks")
    _stub.get_axon_ntff_profile_hook = lambda: None
    try:
        import antenv

        sys.modules["antenv.axon_hooks"] = _stub
        antenv.axon_hooks = _stub
    except Exception:
        _pkg = types.ModuleType("antenv")
        _pkg.axon_hooks = _stub
        sys.modules["antenv"] = _pkg
        sys.modules["antenv.axon_hooks"] = _stub

import concourse.bass as bass
import concourse.bacc as bacc
import concourse.mybir as mybir
from concourse.tile import TileContext
from concourse.masks import make_identity
from concourse.bass_utils import run_bass_kernel_spmd

P = 128
B = 8
L = 4096
D = 1024
K = 4
SB = 512  # L-superblock
G = D // P
NSB = L // SB
TPB = SB // P  # 128-l tiles per superblock
PAD = 16  # leading zero rows on x (xbar needs 16-aligned column offsets)

ALU = mybir.AluOpType

CFG = {
    "ew_groups": (6, 7),        # elementwise (non-PE-MAC) groups
    # xbar chunk sizes in l (sum = L); finer early chunks start compute
    # sooner. chunk_emit[i] = superblock body before which chunk i is
    # emitted (-1 = prologue); all DMAs ride the single SP FIFO so the
    # scheduler's enforced order matches readiness.
    "xbar_chunks": (1024, 1024, 1024, 1024),
    "chunk_emit": (-1, -1, -1, -1),
    "g_orders": {
        "0": (6, 0, 7, 1, 2, 3, 4, 5),
        "1": (0, 1, 2, 3, 6, 7, 4, 5),
        "2": (0, 1, 2, 3, 6, 7, 4, 5),
        "3": (0, 1, 2, 3, 6, 7, 4, 5),
    },
    "psA": 512,                 # first psum tile width (d cols of PE groups)
    "psA_bufs": 3,
    "psB_bufs": 2,
    "psE_bufs": 2,
    "yt7_bufs": 3,
    "yout_bufs": 8,
}


def build_conv_nc(cfg=CFG):
    f32 = mybir.dt.float32
    bf16 = mybir.dt.bfloat16
    ew = tuple(cfg["ew_groups"])
    pe_groups = [g for g in range(G) if g not in ew]
    M = len(pe_groups)
    MD = M * P           # total d-cols produced by PE MAC per l-tile
    # evac layout assumes PE groups occupy the leading d-columns in order
    assert pe_groups + list(ew) == list(range(G))

    nc = bacc.Bacc("TRN2", target_bir_lowering=False)
    # xt col = PAD + l: xbar output column offsets must be 16-aligned, so
    # the causal halo is PAD=16 memset zero cols in front
    x_d = nc.dram_tensor("x", [L, D], bf16, kind="ExternalInput")
    wb_d = nc.dram_tensor("wb", [P, G * K + G], f32, kind="ExternalInput")
    y_d = nc.dram_tensor("y", [L, D], bf16, kind="ExternalOutput")

    with TileContext(nc) as tc:
        with (
            tc.tile_pool(name="const", bufs=1) as constp,
            tc.tile_pool(name="yt7", bufs=cfg["yt7_bufs"]) as yt7p,
            tc.tile_pool(name="yout", bufs=cfg["yout_bufs"]) as youtp,
            tc.tile_pool(name="ps", bufs=cfg["psA_bufs"], space="PSUM") as psp,
            tc.tile_pool(name="ps_e", bufs=cfg["psE_bufs"], space="PSUM") as pse,
        ):
            # --- one merged const DMA (two would interleave into the xbar
            # lane rotation, each costing ~2.2us of in-stream start delay) ---
            wb = constp.tile([P, G * K + G], f32)
            nc.sync.dma_start(out=wb, in_=wb_d[:, :])
            wcols = wb[:, 0 : G * K]
            bcol = wb[:, G * K : G * K + G]

            # --- whole-L transposed x tiles (16 zero halo cols memset) ---
            xt = [
                constp.tile([P, PAD + L], bf16, name=f"xt{g}") for g in range(G)
            ]
            for g in range(G):
                nc.vector.memset(xt[g][:, 0:PAD], 0.0)

            chunks = []
            off = 0
            for csz in cfg["xbar_chunks"]:
                chunks.append((off, csz))
                off += csz
            assert off == L

            default_order = list(ew) + pe_groups
            g_orders = cfg.get("g_orders") or {}

            def emit_chunk(ci):
                lo, csz = chunks[ci]
                for g in g_orders.get(str(ci), default_order):
                    nc.sync.dma_start_transpose(
                        out=xt[g][:, PAD + lo : PAD + lo + csz],
                        in_=x_d[lo : lo + csz, g * P : (g + 1) * P],
                    )

            emit_by_body = {}
            for ci, body in enumerate(cfg["chunk_emit"]):
                emit_by_body.setdefault(body, []).append(ci)
            for ci in emit_by_body.get(-1, []):
                emit_chunk(ci)

            # --- constants built on device (PE+DVE idle during xbar fill) ---
            identf = constp.tile([P, P], f32)
            make_identity(nc, identf)
            identb = constp.tile([P, P], bf16)
            nc.vector.tensor_copy(out=identb, in_=identf[:, :])
            # prewarm the ACT Identity table
            warm = constp.tile([P, 1], f32)
            nc.scalar.activation(
                warm[:, :],
                identf[:, 0:1],
                mybir.ActivationFunctionType.Identity,
                bias=0.0,
                scale=1.0,
            )
            # bf16 diag(w_k) rhs tiles for the natural-out MAC
            dwb = constp.tile([P, M * K, P], bf16)
            for mi, g in enumerate(pe_groups):
                for k in range(K):
                    nc.vector.tensor_scalar_mul(
                        dwb[:, mi * K + k, :],
                        identf[:, :],
                        wcols[:, g * K + k : g * K + k + 1],
                    )
            # partition-broadcast bias band for PE groups, built on device:
            # bband[p, mi*128+j] = b[pe_groups[mi]*128+j]  via ones^T @ diag(b)
            onesb = constp.tile([P, P], bf16)
            nc.vector.memset(onesb[:, :], 1.0)
            db = constp.tile([P, M, P], bf16)
            for mi, g in enumerate(pe_groups):
                nc.vector.tensor_scalar_mul(
                    db[:, mi, :], identf[:, :], bcol[:, g : g + 1]
                )
            bband = constp.tile([P, MD], bf16)
            bb_ps = psp.tile([P, MD], f32, name="ps")
            for mi in range(M):
                nc.tensor.matmul(
                    bb_ps[:, mi * P : (mi + 1) * P], onesb[:, :], db[:, mi, :],
                    start=True, stop=True,
                )
            nc.scalar.copy(out=bband[:, :], in_=bb_ps[:, :])

            # --- elementwise tree for superblock s, pipelined one sb ahead.
            # a-pass on ACT, scaled copies on DVE (tensor_scalar @4x bf16),
            # adds split Pool/DVE; PE transposes back during sb s.
            def emit_tree(s):
                yts = {}
                for g in ew:
                    gb = s * SB + PAD - 3  # xt col of x[l-3] for l = s*SB
                    a = yt7p.tile([P, SB], bf16, tag=f"a{g}", name=f"a{g}")
                    c = yt7p.tile([P, SB], bf16, tag=f"c{g}", name=f"c{g}")
                    d_ = yt7p.tile([P, SB], bf16, tag=f"d{g}", name=f"d{g}")
                    e0 = yt7p.tile([P, SB], bf16, tag=f"e{g}", name=f"e{g}")
                    yt = yt7p.tile([P, SB], bf16, tag=f"yt{g}", name=f"yt{g}")
                    # a = x[l]*w3 + b (ACT); c,d,e0 = scaled copies (DVE 4x)
                    nc.scalar.activation(
                        a[:, :], xt[g][:, gb + 3 : gb + 3 + SB],
                        mybir.ActivationFunctionType.Identity,
                        bias=bcol[:, g : g + 1],
                        scale=wcols[:, g * K + 3 : g * K + 4],
                    )
                    nc.vector.tensor_scalar_mul(
                        c[:, :], xt[g][:, gb + 2 : gb + 2 + SB],
                        wcols[:, g * K + 2 : g * K + 3],
                    )
                    nc.vector.tensor_scalar_mul(
                        d_[:, :], xt[g][:, gb + 1 : gb + 1 + SB],
                        wcols[:, g * K + 1 : g * K + 2],
                    )
                    nc.vector.tensor_scalar_mul(
                        e0[:, :], xt[g][:, gb : gb + SB],
                        wcols[:, g * K : g * K + 1],
                    )
                    # Pool: c += d ; DVE: a += e0 ; Pool: yt = a + c
                    nc.gpsimd.tensor_tensor(
                        out=c[:, :], in0=c[:, :], in1=d_[:, :], op=ALU.add
                    )
                    nc.vector.tensor_tensor(
                        out=a[:, :], in0=a[:, :], in1=e0[:, :], op=ALU.add
                    )
                    nc.gpsimd.tensor_tensor(
                        out=yt[:, :], in0=a[:, :], in1=c[:, :], op=ALU.add
                    )
                    yts[g] = yt
                return yts

            # --- main loop ---
            trees = emit_tree(0)
            for s in range(NSB):
                last = s == NSB - 1
                for ci in emit_by_body.get(s, []):
                    emit_chunk(ci)
                y_tile = youtp.tile([P, TPB, D], bf16, name="y_tile")
                yv = y_d[s * SB : (s + 1) * SB, :].rearrange(
                    "(t p) d -> p t d", p=P
                )
                yts, trees = trees, (emit_tree(s + 1) if not last else None)

                ps_e = pse.tile([P, len(ew), SB], bf16, name="ps_e")
                for tl in range(TPB):
                    t = s * TPB + tl
                    ps1 = psp.tile([P, MD], f32, name="ps")
                    for mi, g in enumerate(pe_groups):
                        o = mi * P
                        for k in range(K):
                            nc.tensor.matmul(
                                ps1[:, o : o + P],
                                xt[g][:, t * P + k + PAD - 3 : t * P + k + PAD - 3 + P],
                                dwb[:, mi * K + k, :],
                                start=(k == 0),
                                stop=(k == K - 1),
                            )
                    # per-t transpose-out of the elementwise groups (PE)
                    for gi, g in enumerate(ew):
                        nc.tensor.transpose(
                            ps_e[:, gi, tl * P : (tl + 1) * P],
                            yts[g][:, tl * P : (tl + 1) * P],
                            identb,
                        )
                    # evac with fused bias (bband) on DVE; e-copies on ACT
                    nc.vector.tensor_tensor(
                        out=y_tile[:, tl, 0:MD],
                        in0=ps1[:, :], in1=bband[:, :], op=ALU.add,
                    )
                    for gi, g in enumerate(ew):
                        nc.scalar.copy(
                            out=y_tile[:, tl, (M + gi) * P : (M + gi + 1) * P],
                            in_=ps_e[:, gi, tl * P : (tl + 1) * P],
                        )
                    # out DMA: drains as soon as this t's evacs land.
                    # out_pair batches two t per DMA (fewer HWDGE slots);
                    # act_outs issues this sb's outs from the ACT ring.
                    oq = nc.scalar if s in cfg.get("act_outs", ()) else nc.sync
                    if cfg.get("out_pair") and s not in (0, NSB - 1):
                        if tl % 2 == 1:
                            oq.dma_start(
                                out=yv[:, tl - 1 : tl + 1, :],
                                in_=y_tile[:, tl - 1 : tl + 1, :],
                            )
                    else:
                        oq.dma_start(out=yv[:, tl, :], in_=y_tile[:, tl, :])
    nc.finalize()
    return nc


def host_prep(w, b):
    w = np.asarray(w, dtype=np.float32).reshape(D, K)
    b = np.asarray(b, dtype=np.float32).reshape(D)
    wb = np.empty((P, G * K + G), dtype=np.float32)
    for g in range(G):
        wb[:, G * K + g] = b[g * P : (g + 1) * P]
        for k in range(K):
            wb[:, g * K + k] = w[g * P : (g + 1) * P, k]
    return {"wb": wb}


_NC_CACHE = {}


def _get_nc():
    key = str(CFG)
    if key not in _NC_CACHE:
        _NC_CACHE[key] = build_conv_nc()
    return _NC_CACHE[key]


def kernel(x, w, b, _trace=False):
    import ml_dtypes

    x = np.asarray(x, dtype=np.float32)
    assert x.shape == (B, L, D), x.shape
    consts = host_prep(w, b)
    nc = _get_nc()
    xs = x.astype(ml_dtypes.bfloat16)
    in_maps = [{"x": np.ascontiguousarray(xs[i]), **consts} for i in range(B)]
    res = run_bass_kernel_spmd(nc, in_maps, core_ids=list(range(B)), trace=_trace)
    y = np.stack(
        [np.asarray(res.results[i]["y"], dtype=np.float32) for i in range(B)], axis=0
    )
    if _trace:
        return y, res
    return y


# revision 3
# speedup vs baseline: 1.0012x; 1.0012x over previous
"""Causal depthwise Conv1d (K=4) for Trainium2, 8 NeuronCores.

Problem: x (B=8, L=4096, D=1024) f32, w (D, 1, 4), b (D,)
  y[n, l, d] = sum_k w[d, 0, k] * x[n, l - 3 + k, d] + b[d]   (zero pad l<0)

Sharding: data-parallel over batch - core i computes batch item i.

Per-core design (bf16 I/O, ~61us cost model vs 82us for the PE-transpose
baseline; DMA-wire serial time 52us is the floor):
  1. In: per d-group g (8 groups of 128 channels), XBAR DMA transposes
     (16x128 tiles, 14ns each, no PE or PSUM involvement) land x directly
     as transposed whole-L SBUF tiles xt_g[128_d, 16+4096_l]; the 16-col
     memset halo handles causal padding (xbar output offsets must be
     16-aligned). Four 1024-l chunks per group, all issued up front on the
     single SP DMA ring (mixing rings stalls the tile scheduler's enforced
     cross-queue DMA order); per-chunk group order tuned so PE-MAC groups
     land just before their MACs and e-groups just before their trees.
  2. MAC for groups 0..5: natural-layout bf16 diag matmuls, 4 taps per
     [128_l, 128_d] block accumulating into a [128, 768] PSUM tile; one
     DVE tensor_tensor per l-tile folds the partition-broadcast bias band
     in while evacuating to bf16 SBUF (no transpose-out for these groups).
  3. Groups 6,7 compute elementwise in transposed space, one sb ahead:
     ACT does w3*x+b, DVE tensor_scalar (4x bf16 mode) makes the three
     scaled copies, Pool/DVE sum them; PE transposes the result back
     per l-tile and ACT copies it into the output tile.
  4. Per-l-tile out DMAs (2KB descriptors) drain each 128-row slice as
     soon as its evacuations land, on the same SP ring after the xbars.

Engine budget per l-tile (steady state): PE 1.39us (24 MAC + 2 T-out
matmuls), DVE 1.38us (wide evac + tree share), Pool 1.11us, ACT 0.92us;
wire 1.63us averaged. Numerics: bf16 in/compute/out, rel err ~3e-3 vs
the 2e-2 gate.
"""

import sys
import types

import numpy as np

try:  # the NTFF profile hook module is absent in some containers
    import antenv.axon_hooks  # noqa: F401
except Exception:
    _stub = types.ModuleType("antenv.axon_hooks")
    _stub.get_axon_ntff_profile_hook = lambda: None
    try:
        import antenv

        sys.modules["antenv.axon_hooks"] = _stub
        antenv.axon_hooks = _stub
    except Exception:
        _pkg = types.ModuleType("antenv")
        _pkg.axon_hooks = _stub
        sys.modules["antenv"] = _pkg
        sys.modules["antenv.axon_hooks"] = _stub

import concourse.bass as bass
import concourse.bacc as bacc
import concourse.mybir as mybir
from concourse.tile import TileContext
from concourse.masks import make_identity
from concourse.bass_utils import run_bass_kernel_spmd

P = 128
B = 8
L = 4096
D = 1024
K = 4
SB = 512  # L-superblock
G = D // P
NSB = L // SB
TPB = SB // P  # 128-l tiles per superblock
PAD = 16  # leading zero rows on x (xbar needs 16-aligned column offsets)

ALU = mybir.AluOpType

CFG = {
    "ew_groups": (6, 7),        # elementwise (non-PE-MAC) groups
    # xbar chunk sizes in l (sum = L); finer early chunks start compute
    # sooner. chunk_emit[i] = superblock body before which chunk i is
    # emitted (-1 = prologue); all DMAs ride the single SP FIFO so the
    # scheduler's enforced order matches readiness.
    "xbar_chunks": (1024, 1024, 1024, 1024),
    "chunk_emit": (-1, -1, -1, -1),
    "g_orders": {
        "0": (6, 0, 7, 1, 2, 3, 4, 5),
        "1": (0, 6, 7, 1, 2, 3, 4, 5),
        "2": (0, 1, 2, 6, 7, 3, 4, 5),
        "3": (0, 1, 6, 7, 2, 3, 4, 5),
    },
    "psA": 512,                 # first psum tile width (d cols of PE groups)
    "psA_bufs": 3,
    "psB_bufs": 2,
    "psE_bufs": 2,
    "yt7_bufs": 3,
    "yout_bufs": 8,
}


def build_conv_nc(cfg=CFG):
    f32 = mybir.dt.float32
    bf16 = mybir.dt.bfloat16
    ew = tuple(cfg["ew_groups"])
    pe_groups = [g for g in range(G) if g not in ew]
    M = len(pe_groups)
    MD = M * P           # total d-cols produced by PE MAC per l-tile
    # evac layout assumes PE groups occupy the leading d-columns in order
    assert pe_groups + list(ew) == list(range(G))

    nc = bacc.Bacc("TRN2", target_bir_lowering=False)
    # xt col = PAD + l: xbar output column offsets must be 16-aligned, so
    # the causal halo is PAD=16 memset zero cols in front
    x_d = nc.dram_tensor("x", [L, D], bf16, kind="ExternalInput")
    wb_d = nc.dram_tensor("wb", [P, G * K + G], f32, kind="ExternalInput")
    y_d = nc.dram_tensor("y", [L, D], bf16, kind="ExternalOutput")

    with TileContext(nc) as tc:
        with (
            tc.tile_pool(name="const", bufs=1) as constp,
            tc.tile_pool(name="yt7", bufs=cfg["yt7_bufs"]) as yt7p,
            tc.tile_pool(name="yout", bufs=cfg["yout_bufs"]) as youtp,
            tc.tile_pool(name="ps", bufs=cfg["psA_bufs"], space="PSUM") as psp,
            tc.tile_pool(name="ps_e", bufs=cfg["psE_bufs"], space="PSUM") as pse,
        ):
            # --- one merged const DMA (two would interleave into the xbar
            # lane rotation, each costing ~2.2us of in-stream start delay) ---
            wb = constp.tile([P, G * K + G], f32)
            nc.sync.dma_start(out=wb, in_=wb_d[:, :])
            wcols = wb[:, 0 : G * K]
            bcol = wb[:, G * K : G * K + G]

            # --- whole-L transposed x tiles (16 zero halo cols memset) ---
            xt = [
                constp.tile([P, PAD + L], bf16, name=f"xt{g}") for g in range(G)
            ]
            for g in range(G):
                nc.vector.memset(xt[g][:, 0:PAD], 0.0)

            chunks = []
            off = 0
            for csz in cfg["xbar_chunks"]:
                chunks.append((off, csz))
                off += csz
            assert off == L

            default_order = list(ew) + pe_groups
            g_orders = cfg.get("g_orders") or {}

            def emit_chunk(ci):
                lo, csz = chunks[ci]
                for g in g_orders.get(str(ci), default_order):
                    nc.sync.dma_start_transpose(
                        out=xt[g][:, PAD + lo : PAD + lo + csz],
                        in_=x_d[lo : lo + csz, g * P : (g + 1) * P],
                    )

            emit_by_body = {}
            for ci, body in enumerate(cfg["chunk_emit"]):
                emit_by_body.setdefault(body, []).append(ci)
            for ci in emit_by_body.get(-1, []):
                emit_chunk(ci)

            # --- constants built on device (PE+DVE idle during xbar fill) ---
            identf = constp.tile([P, P], f32)
            make_identity(nc, identf)
            identb = constp.tile([P, P], bf16)
            nc.vector.tensor_copy(out=identb, in_=identf[:, :])
            # prewarm the ACT Identity table
            warm = constp.tile([P, 1], f32)
            nc.scalar.activation(
                warm[:, :],
                identf[:, 0:1],
                mybir.ActivationFunctionType.Identity,
                bias=0.0,
                scale=1.0,
            )
            # bf16 diag(w_k) rhs tiles for the natural-out MAC
            dwb = constp.tile([P, M * K, P], bf16)
            for mi, g in enumerate(pe_groups):
                for k in range(K):
                    nc.vector.tensor_scalar_mul(
                        dwb[:, mi * K + k, :],
                        identf[:, :],
                        wcols[:, g * K + k : g * K + k + 1],
                    )
            # partition-broadcast bias band for PE groups, built on device:
            # bband[p, mi*128+j] = b[pe_groups[mi]*128+j]  via ones^T @ diag(b)
            onesb = constp.tile([P, P], bf16)
            nc.vector.memset(onesb[:, :], 1.0)
            db = constp.tile([P, M, P], bf16)
            for mi, g in enumerate(pe_groups):
                nc.vector.tensor_scalar_mul(
                    db[:, mi, :], identf[:, :], bcol[:, g : g + 1]
                )
            bband = constp.tile([P, MD], bf16)
            bb_ps = psp.tile([P, MD], f32, name="ps")
            for mi in range(M):
                nc.tensor.matmul(
                    bb_ps[:, mi * P : (mi + 1) * P], onesb[:, :], db[:, mi, :],
                    start=True, stop=True,
                )
            nc.scalar.copy(out=bband[:, :], in_=bb_ps[:, :])

            # --- elementwise tree for superblock s, pipelined one sb ahead.
            # a-pass on ACT, scaled copies on DVE (tensor_scalar @4x bf16),
            # adds split Pool/DVE; PE transposes back during sb s.
            def emit_tree(s):
                yts = {}
                for g in ew:
                    gb = s * SB + PAD - 3  # xt col of x[l-3] for l = s*SB
                    a = yt7p.tile([P, SB], bf16, tag=f"a{g}", name=f"a{g}")
                    c = yt7p.tile([P, SB], bf16, tag=f"c{g}", name=f"c{g}")
                    d_ = yt7p.tile([P, SB], bf16, tag=f"d{g}", name=f"d{g}")
                    e0 = yt7p.tile([P, SB], bf16, tag=f"e{g}", name=f"e{g}")
                    yt = yt7p.tile([P, SB], bf16, tag=f"yt{g}", name=f"yt{g}")
                    # a = x[l]*w3 + b (ACT); c,d,e0 = scaled copies (DVE 4x)
                    nc.scalar.activation(
                        a[:, :], xt[g][:, gb + 3 : gb + 3 + SB],
                        mybir.ActivationFunctionType.Identity,
                        bias=bcol[:, g : g + 1],
                        scale=wcols[:, g * K + 3 : g * K + 4],
                    )
                    nc.vector.tensor_scalar_mul(
                        c[:, :], xt[g][:, gb + 2 : gb + 2 + SB],
                        wcols[:, g * K + 2 : g * K + 3],
                    )
                    nc.vector.tensor_scalar_mul(
                        d_[:, :], xt[g][:, gb + 1 : gb + 1 + SB],
                        wcols[:, g * K + 1 : g * K + 2],
                    )
                    nc.vector.tensor_scalar_mul(
                        e0[:, :], xt[g][:, gb : gb + SB],
                        wcols[:, g * K : g * K + 1],
                    )
                    # Pool: c += d ; DVE: a += e0 ; Pool: yt = a + c
                    nc.gpsimd.tensor_tensor(
                        out=c[:, :], in0=c[:, :], in1=d_[:, :], op=ALU.add
                    )
                    nc.vector.tensor_tensor(
                        out=a[:, :], in0=a[:, :], in1=e0[:, :], op=ALU.add
                    )
                    nc.gpsimd.tensor_tensor(
                        out=yt[:, :], in0=a[:, :], in1=c[:, :], op=ALU.add
                    )
                    yts[g] = yt
                return yts

            # --- main loop ---
            trees = emit_tree(0)
            for s in range(NSB):
                last = s == NSB - 1
                for ci in emit_by_body.get(s, []):
                    emit_chunk(ci)
                y_tile = youtp.tile([P, TPB, D], bf16, name="y_tile")
                yv = y_d[s * SB : (s + 1) * SB, :].rearrange(
                    "(t p) d -> p t d", p=P
                )
                yts, trees = trees, (emit_tree(s + 1) if not last else None)

                ps_e = pse.tile([P, len(ew), SB], bf16, name="ps_e")
                for tl in range(TPB):
                    t = s * TPB + tl
                    ps1 = psp.tile([P, MD], f32, name="ps")
                    for mi, g in enumerate(pe_groups):
                        o = mi * P
                        for k in range(K):
                            nc.tensor.matmul(
                                ps1[:, o : o + P],
                                xt[g][:, t * P + k + PAD - 3 : t * P + k + PAD - 3 + P],
                                dwb[:, mi * K + k, :],
                                start=(k == 0),
                                stop=(k == K - 1),
                            )
                    # per-t transpose-out of the elementwise groups (PE)
                    for gi, g in enumerate(ew):
                        nc.tensor.transpose(
                            ps_e[:, gi, tl * P : (tl + 1) * P],
                            yts[g][:, tl * P : (tl + 1) * P],
                            identb,
                        )
                    # evac with fused bias (bband) on DVE; e-copies on ACT
                    nc.vector.tensor_tensor(
                        out=y_tile[:, tl, 0:MD],
                        in0=ps1[:, :], in1=bband[:, :], op=ALU.add,
                    )
                    for gi, g in enumerate(ew):
                        nc.scalar.copy(
                            out=y_tile[:, tl, (M + gi) * P : (M + gi + 1) * P],
                            in_=ps_e[:, gi, tl * P : (tl + 1) * P],
                        )
                    # out DMA: drains as soon as this t's evacs land.
                    # out_pair batches two t per DMA (fewer HWDGE slots);
                    # act_outs issues this sb's outs from the ACT ring.
                    oq = nc.scalar if s in cfg.get("act_outs", ()) else nc.sync
                    if cfg.get("out_pair") and s not in (0, NSB - 1):
                        if tl % 2 == 1:
                            oq.dma_start(
                                out=yv[:, tl - 1 : tl + 1, :],
                                in_=y_tile[:, tl - 1 : tl + 1, :],
                            )
                    else:
                        oq.dma_start(out=yv[:, tl, :], in_=y_tile[:, tl, :])
    nc.finalize()
    return nc


def host_prep(w, b):
    w = np.asarray(w, dtype=np.float32).reshape(D, K)
    b = np.asarray(b, dtype=np.float32).reshape(D)
    wb = np.empty((P, G * K + G), dtype=np.float32)
    for g in range(G):
        wb[:, G * K + g] = b[g * P : (g + 1) * P]
        for k in range(K):
            wb[:, g * K + k] = w[g * P : (g + 1) * P, k]
    return {"wb": wb}


_NC_CACHE = {}


def _get_nc():
    key = str(CFG)
    if key not in _NC_CACHE:
        _NC_CACHE[key] = build_conv_nc()
    return _NC_CACHE[key]


def kernel(x, w, b, _trace=False):
    import ml_dtypes

    x = np.asarray(x, dtype=np.float32)
    assert x.shape == (B, L, D), x.shape
    consts = host_prep(w, b)
    nc = _get_nc()
    xs = x.astype(ml_dtypes.bfloat16)
    in_maps = [{"x": np.ascontiguousarray(xs[i]), **consts} for i in range(B)]
    res = run_bass_kernel_spmd(nc, in_maps, core_ids=list(range(B)), trace=_trace)
    y = np.stack(
        [np.asarray(res.results[i]["y"], dtype=np.float32) for i in range(B)], axis=0
    )
    if _trace:
        return y, res
    return y
